# revision 15
# baseline (speedup 1.0000x reference)
"""Trainium2 Bass kernel for the contrastive-loss forward pass.

Math (forward value of the reference):
    zn = z / max(||z||, eps);  yn = y_interp / max(||y_interp||, eps)
    sim = (zn @ yn^T) * exp(-log_tau)          # /tau on both sides
    out[i] = clip(log(sum_j exp(sim[i,j]) / B) - sim[i,i], -5, 15)
(the reference's `exp(slogits - stop_gradient(slogits)) - 1` term is exactly 0
in the forward pass, so out == dummy_ce clipped.)

Sharding: data-parallel over batch rows across 8 NeuronCores.  Each core
computes its own hy shard, the shards are all-gathered on-chip
(collective_compute AllGather through DRAM bounce buffers), and each core
computes its [1024, 8192] block of sim fused with exp/row-sum so the BxB
matrix never touches DRAM.
"""

import sys

if "/opt/trn_rl_repo" not in sys.path:
    sys.path.insert(0, "/opt/trn_rl_repo")

from contextlib import ExitStack

import numpy as np

import concourse.bacc as bacc
import concourse.bass as bass
import concourse.mybir as mybir
import concourse.tile as tile
from concourse.bass_utils import run_bass_kernel_spmd
from concourse.masks import make_identity

N_CORES = 8
B = 8192
M = 256                # landmark count == feature dim
BL = B // N_CORES      # 1024 batch rows per core
P = 128
NT = BL // P           # 8 row-tiles per core
NB = 512               # matmul moving free-dim chunk
F32 = mybir.dt.float32
BF16 = mybir.dt.bfloat16
I32 = mybir.dt.int32
AF = mybir.ActivationFunctionType
OP = mybir.AluOpType
EPS = 1e-12


def _emit(ctx: ExitStack, tc, nc, z_d, t_d, e_d, lt_d, E_d, L_d, out_d, stage=99, rep=0):
    cpool = ctx.enter_context(tc.tile_pool(name=f"c{rep}", bufs=1))
    pst = ctx.enter_context(tc.tile_pool(name=f"pst{rep}", bufs=3, space="PSUM"))
    dram = ctx.enter_context(tc.tile_pool(name=f"dram{rep}", bufs=1, space="DRAM"))
    ypool = ctx.enter_context(tc.tile_pool(name=f"y{rep}", bufs=4))
    gpool = ctx.enter_context(tc.tile_pool(name=f"g{rep}", bufs=NT))

    # ---- constants ----
    J_i = cpool.tile([P, M], I32, tag="J_i")
    nc.gpsimd.iota(J_i[:], pattern=[[1, M]], base=0, channel_multiplier=0)
    J_b = cpool.tile([P, M], F32, tag="J_b")
    nc.vector.tensor_copy(J_b[:], J_i[:])
    ones = cpool.tile([P, M], F32, tag="ones")
    nc.vector.memset(ones[:], 1.0)
    idn_f = cpool.tile([P, P], F32, tag="idn_f")
    make_identity(nc, idn_f[:])
    idn_b = cpool.tile([P, P], BF16, tag="idn_b")
    make_identity(nc, idn_b[:])

    L_sb = cpool.tile([1, M], F32, tag="L_sb")
    nc.sync.dma_start(L_sb[:], L_d[:])
    L_b = cpool.tile([P, M], F32, tag="L_b")
    nc.gpsimd.partition_broadcast(L_b[:], L_sb[:])

    lt_sb = cpool.tile([1, 1], F32, tag="lt_sb")
    nc.sync.dma_start(lt_sb[:], lt_d[:])
    it1 = cpool.tile([1, 1], F32, tag="it1")
    nc.scalar.activation(it1[:], lt_sb[:], AF.Exp, scale=-1.0)  # exp(-log_tau)
    inv_t2 = cpool.tile([P, 1], F32, tag="inv_t2")
    nc.gpsimd.partition_broadcast(inv_t2[:], it1[:])
    nit2 = cpool.tile([P, 1], F32, tag="nit2")
    nc.vector.tensor_scalar_mul(nit2[:], inv_t2[:], -1.0)

    # ---- E, E^T and the suffix-sum table Tt[r, k] = sum_{j>=r} E[k, j] ----
    E_sb = []
    for c in range(2):
        esb = cpool.tile([P, M], F32, tag=f"E_sb{c}")
        nc.sync.dma_start(esb[:], E_d[c * P:(c + 1) * P, :])
        E_sb.append(esb)
    ET = [cpool.tile([P, M], F32, tag=f"ET{c}", name=f"ET{c}") for c in range(2)]
    for a in range(2):
        for b in range(2):
            pt = pst.tile([P, P], F32, tag="tp")
            nc.tensor.transpose(pt[:], E_sb[a][:, b * P:(b + 1) * P], idn_f[:])
            nc.vector.tensor_copy(ET[b][:, a * P:(a + 1) * P], pt[:])
    u2t = [cpool.tile([P, M], F32, tag=f"u2t{c}", name=f"u2t{c}") for c in range(2)]
    for jc in range(2):
        # u2t[jc][p, r] = 1.0 if (jc*128 + p) >= r else 0.0
        nc.gpsimd.affine_select(
            out=u2t[jc][:], in_=ones[:], base=jc * P, channel_multiplier=1,
            pattern=[[-1, M]], compare_op=OP.is_ge, fill=0.0,
        )
    Tt_d = dram.tile([M, M], F32)
    for rc in range(2):
        ps = pst.tile([P, M], F32, tag="tp")
        for jc in range(2):
            nc.tensor.matmul(
                ps[:], lhsT=u2t[jc][:, rc * P:(rc + 1) * P], rhs=ET[jc][:],
                start=(jc == 0), stop=(jc == 1),
            )
        tts = ypool.tile([P, M], F32, tag="tts")
        nc.vector.tensor_copy(tts[:], ps[:])
        nc.sync.dma_start(Tt_d[rc * P:(rc + 1) * P, :], tts[:])

    def _bail():
        zo = ypool.tile([P, 1], F32, tag="zo")
        nc.vector.memset(zo[:], 0.0)
        for it_ in range(NT):
            nc.sync.dma_start(out_d[it_ * P:(it_ + 1) * P, :], zo[:])

    if stage <= 0:
        _bail()
        return

    # ---- phase Y: per-row interpolated/censored embedding, normalized ----
    hyT = [cpool.tile([P, BL], BF16, tag=f"hyT{c}", name=f"hyT{c}") for c in range(2)]
    HB = NT // 2 * P
    cc_in_h = [dram.tile([2, P, HB], BF16, tag=f"cci{h}", name=f"cci{h}")
               for h in range(2)]
    cc_out_h = [dram.tile([N_CORES, 2, P, HB], BF16, addr_space="Shared",
                          tag=f"cco{h}", name=f"cco{h}") for h in range(2)]
    ynb_keep = []
    for it in range(NT):
        tcol = ypool.tile([P, 1], F32, tag="tcol")
        nc.sync.dma_start(tcol[:], t_d[it])
        ecol = ypool.tile([P, 1], F32, tag="ecol")
        nc.sync.dma_start(ecol[:], e_d[it])
        if 10 <= stage <= 11:
            continue

        scr = ypool.tile([P, M], F32, tag="scr")
        idx = ypool.tile([P, 1], F32, tag="idx")
        # searchsorted(L, t, side='left') == count of L[j] < t
        nc.vector.scalar_tensor_tensor(
            out=scr[:], in0=L_b[:], scalar=tcol[:], in1=ones[:],
            op0=OP.is_lt, op1=OP.mult, accum_out=idx[:],
        )
        nc.vector.tensor_scalar(
            out=idx[:], in0=idx[:], scalar1=1.0, scalar2=float(M - 1),
            op0=OP.max, op1=OP.min,
        )
        idxm1 = ypool.tile([P, 1], F32, tag="idxm1")
        nc.vector.tensor_scalar_add(idxm1[:], idx[:], -1.0)
        idx_i = ypool.tile([P, 1], I32, tag="idx_i")
        nc.vector.tensor_copy(idx_i[:], idx[:])
        idxm1_i = ypool.tile([P, 1], I32, tag="idxm1_i")
        nc.vector.tensor_copy(idxm1_i[:], idxm1[:])
        if 10 <= stage <= 12:
            continue

        E_hi = ypool.tile([P, M], F32, tag="E_hi")
        nc.gpsimd.indirect_dma_start(
            out=E_hi[:], out_offset=None, in_=E_d[:],
            in_offset=bass.IndirectOffsetOnAxis(ap=idx_i[:, :1], axis=0),
        )
        E_lo = ypool.tile([P, M], F32, tag="E_lo")
        nc.gpsimd.indirect_dma_start(
            out=E_lo[:], out_offset=None, in_=E_d[:],
            in_offset=bass.IndirectOffsetOnAxis(ap=idxm1_i[:, :1], axis=0),
        )
        Trow = ypool.tile([P, M], F32, tag="Trow")
        nc.gpsimd.indirect_dma_start(
            out=Trow[:], out_offset=None, in_=Tt_d[:],
            in_offset=bass.IndirectOffsetOnAxis(ap=idx_i[:, :1], axis=0),
        )
        if 10 <= stage <= 13:
            continue

        # L_hi/L_lo: sum_j (J == idx) * L[j]  (fused one-hot dot)
        Lhi = ypool.tile([P, 1], F32, tag="Lhi")
        Llo = ypool.tile([P, 1], F32, tag="Llo")
        nc.vector.scalar_tensor_tensor(
            out=scr[:], in0=J_b[:], scalar=idx[:], in1=L_b[:],
            op0=OP.is_equal, op1=OP.mult, accum_out=Lhi[:],
        )
        nc.vector.scalar_tensor_tensor(
            out=scr[:], in0=J_b[:], scalar=idxm1[:], in1=L_b[:],
            op0=OP.is_equal, op1=OP.mult, accum_out=Llo[:],
        )
        if 10 <= stage <= 14:
            continue

        # w = (t - L_lo) / (L_hi - L_lo)
        dd = ypool.tile([P, 1], F32, tag="dd")
        nc.vector.tensor_tensor(out=dd[:], in0=Lhi[:], in1=Llo[:], op=OP.subtract)
        rdd = ypool.tile([P, 1], F32, tag="rdd")
        nc.vector.reciprocal(rdd[:], dd[:])
        td = ypool.tile([P, 1], F32, tag="td")
        nc.vector.tensor_tensor(out=td[:], in0=tcol[:], in1=Llo[:], op=OP.subtract)
        w = ypool.tile([P, 1], F32, tag="w")
        nc.vector.tensor_tensor(out=w[:], in0=td[:], in1=rdd[:], op=OP.mult)
        if 10 <= stage <= 15:
            continue

        # censor scale = (1 - e) / (M - idx)
        den = ypool.tile([P, 1], F32, tag="den")
        nc.vector.tensor_scalar(
            out=den[:], in0=idx[:], scalar1=-1.0, scalar2=float(M),
            op0=OP.mult, op1=OP.add,
        )
        rden = ypool.tile([P, 1], F32, tag="rden")
        nc.vector.reciprocal(rden[:], den[:])
        ome = ypool.tile([P, 1], F32, tag="ome")
        nc.vector.tensor_scalar(
            out=ome[:], in0=ecol[:], scalar1=-1.0, scalar2=1.0,
            op0=OP.mult, op1=OP.add,
        )
        cs = ypool.tile([P, 1], F32, tag="cs")
        nc.vector.tensor_tensor(out=cs[:], in0=ome[:], in1=rden[:], op=OP.mult)
        if 10 <= stage <= 16:
            continue

        # y_pre = e*(E_lo + w*(E_hi-E_lo)) + cs*Trow
        edt = ypool.tile([P, M], F32, tag="edt")
        nc.gpsimd.tensor_tensor(out=edt[:], in0=E_hi[:], in1=E_lo[:], op=OP.subtract)
        lerp = ypool.tile([P, M], F32, tag="lerp")
        nc.vector.scalar_tensor_tensor(
            out=lerp[:], in0=edt[:], scalar=w[:], in1=E_lo[:],
            op0=OP.mult, op1=OP.add,
        )
        csr = ypool.tile([P, M], F32, tag="csr")
        nc.gpsimd.tensor_scalar(out=csr[:], in0=Trow[:], scalar1=cs[:], scalar2=None, op0=OP.mult)
        ypre = ypool.tile([P, M], F32, tag="ypre")
        nc.vector.scalar_tensor_tensor(
            out=ypre[:], in0=lerp[:], scalar=ecol[:], in1=csr[:],
            op0=OP.mult, op1=OP.add,
        )
        if 10 <= stage <= 17:
            continue

        # normalize rows -> bf16
        ssq = ypool.tile([P, 1], F32, tag="ssq")
        nc.vector.scalar_tensor_tensor(
            out=scr[:], in0=ypre[:], scalar=1.0, in1=ypre[:],
            op0=OP.mult, op1=OP.mult, accum_out=ssq[:],
        )
        nrm = ypool.tile([P, 1], F32, tag="nrm")
        nc.scalar.activation(nrm[:], ssq[:], AF.Sqrt)
        nc.vector.tensor_scalar_max(nrm[:], nrm[:], EPS)
        rn = ypool.tile([P, 1], F32, tag="rn")
        nc.vector.reciprocal(rn[:], nrm[:])
        ynb = gpool.tile([P, M], BF16, tag="ynb")
        nc.vector.tensor_scalar(out=ynb[:], in0=ypre[:], scalar1=rn[:], scalar2=None, op0=OP.mult)
        ynb_keep.append(ynb)
        if 10 <= stage <= 18:
            continue

        for c in range(2):
            pt = pst.tile([P, P], BF16, tag="tp")
            nc.tensor.transpose(pt[:], ynb[:, c * P:(c + 1) * P], idn_b[:])
            nc.vector.tensor_copy(hyT[c][:, it * P:(it + 1) * P], pt[:])

        if stage > 2 and it in (NT // 2 - 1, NT - 1):
            # half of the shard is transposed: kick off its all-gather now
            h = 0 if it == NT // 2 - 1 else 1
            HB = NT // 2 * P  # 512 columns per half
            for c in range(2):
                nc.sync.dma_start(cc_in_h[h][c], hyT[c][:, h * HB:(h + 1) * HB])
            nc.gpsimd.collective_compute(
                "AllGather", OP.bypass, replica_groups=[list(range(N_CORES))],
                ins=[cc_in_h[h][:].opt()], outs=[cc_out_h[h][:].opt()],
            )

    if stage <= 1 or (10 <= stage < 20):
        _bail()
        return

    if stage <= 2.5 and stage <= 2:
        _bail()
        return

    if stage <= 2.75:
        _bail()
        return

    # ---- phase Z (overlaps the collective): normalized z, g = <zn, yn> ----
    hzT = [cpool.tile([P, BL], BF16, tag=f"hzT{c}", name=f"hzT{c}") for c in range(2)]
    g_list = []
    for it in range(NT):
        zt = ypool.tile([P, M], F32, tag="zt")
        nc.sync.dma_start(zt[:], z_d[it * P:(it + 1) * P, :])
        scr2 = ypool.tile([P, M], F32, tag="scr2")
        ssz = ypool.tile([P, 1], F32, tag="ssz")
        nc.vector.scalar_tensor_tensor(
            out=scr2[:], in0=zt[:], scalar=1.0, in1=zt[:],
            op0=OP.mult, op1=OP.mult, accum_out=ssz[:],
        )
        nrmz = ypool.tile([P, 1], F32, tag="nrmz")
        nc.scalar.activation(nrmz[:], ssz[:], AF.Sqrt)
        nc.vector.tensor_scalar_max(nrmz[:], nrmz[:], EPS)
        rz = ypool.tile([P, 1], F32, tag="rz")
        nc.vector.reciprocal(rz[:], nrmz[:])
        znb = ypool.tile([P, M], BF16, tag="znb")
        nc.vector.tensor_scalar(out=znb[:], in0=zt[:], scalar1=rz[:], scalar2=None, op0=OP.mult)

        g_it = gpool.tile([P, 1], F32, tag="g")
        scrg = ypool.tile([P, M], F32, tag="scrg")
        nc.vector.scalar_tensor_tensor(
            out=scrg[:], in0=znb[:], scalar=1.0, in1=ynb_keep[it][:],
            op0=OP.mult, op1=OP.mult, accum_out=g_it[:],
        )
        g_list.append(g_it)

        for c in range(2):
            pt = pst.tile([P, P], BF16, tag="tp")
            nc.tensor.transpose(pt[:], znb[:, c * P:(c + 1) * P], idn_b[:])
            nc.vector.tensor_copy(hzT[c][:, it * P:(it + 1) * P], pt[:])

    if stage <= 3:
        _bail()
        return

    # ---- bring the gathered hy back to SBUF ----
    hyF = [cpool.tile([P, N_CORES, BL], BF16, tag=f"hyF{c}", name=f"hyF{c}") for c in range(2)]
    for h in range(2):
        for c in range(2):
            for r in range(N_CORES):
                nc.sync.dma_start(hyF[c][:, r, h * HB:(h + 1) * HB],
                                  cc_out_h[h][r, c])

    if stage <= 4:
        # load hyF but skip main loop
        _bail()
        return

    # ---- main fused matmul + exp + row-sum ----
    mps = ctx.enter_context(tc.tile_pool(name=f"mps{rep}", bufs=2, space="PSUM"))
    msc = ctx.enter_context(tc.tile_pool(name=f"msc{rep}", bufs=2))
    apool = ctx.enter_context(tc.tile_pool(name=f"a{rep}", bufs=2))
    for it in range(NT):
        acc = apool.tile([P, N_CORES], F32, tag="acc")
        for g in range(N_CORES):
            ps = mps.tile([P, BL], F32, tag="ps")
            for nb in range(BL // NB):
                for c in range(2):
                    nc.tensor.matmul(
                        ps[:, nb * NB:(nb + 1) * NB],
                        lhsT=hzT[c][:, it * P:(it + 1) * P],
                        rhs=hyF[c][:, g, nb * NB:(nb + 1) * NB],
                        start=(c == 0), stop=(c == 1),
                    )
            sc = msc.tile([P, BL], BF16, tag="esc")
            nc.scalar.activation(
                out=sc[:], in_=ps[:], func=AF.Exp, scale=inv_t2[:],
                accum_out=acc[:, g:g + 1],
            )
        rsum = apool.tile([P, 1], F32, tag="rsum")
        nc.vector.reduce_sum(out=rsum[:], in_=acc[:], axis=mybir.AxisListType.X)
        lr = apool.tile([P, 1], F32, tag="lr")
        nc.scalar.activation(lr[:], rsum[:], AF.Ln, scale=1.0 / B)
        of = apool.tile([P, 1], F32, tag="of")
        nc.vector.scalar_tensor_tensor(
            out=of[:], in0=g_list[it][:], scalar=nit2[:], in1=lr[:],
            op0=OP.mult, op1=OP.add,
        )
        nc.vector.tensor_scalar(
            out=of[:], in0=of[:], scalar1=-5.0, scalar2=15.0, op0=OP.max, op1=OP.min
        )
        nc.sync.dma_start(out_d[it * P:(it + 1) * P, :], of[:])


def build(stage=99, reps=1):
    nc = bacc.Bacc("TRN2", target_bir_lowering=False, debug=False,
                   num_devices=N_CORES)
    z_d = nc.dram_tensor("z", [BL, M], F32, kind="ExternalInput").ap()
    t_d = nc.dram_tensor("t", [NT, P, 1], F32, kind="ExternalInput").ap()
    e_d = nc.dram_tensor("e", [NT, P, 1], F32, kind="ExternalInput").ap()
    lt_d = nc.dram_tensor("log_tau", [1, 1], F32, kind="ExternalInput").ap()
    E_d = nc.dram_tensor("emb", [M, M], F32, kind="ExternalInput").ap()
    L_d = nc.dram_tensor("lm", [1, M], F32, kind="ExternalInput").ap()
    out_d = nc.dram_tensor("out", [BL, 1], F32, kind="ExternalOutput").ap()
    with tile.TileContext(nc) as tc:
        for rep in range(reps):
            with ExitStack() as ctx:
                _emit(ctx, tc, nc, z_d, t_d, e_d, lt_d, E_d, L_d, out_d,
                      stage=stage, rep=rep)
    nc.compile()
    return nc


def make_in_maps(inputs):
    z = np.asarray(inputs["z"], dtype=np.float32)
    t = np.asarray(inputs["t"], dtype=np.float32)
    e = np.asarray(inputs["e"], dtype=np.float32)
    lt = np.asarray(inputs["log_tau"], dtype=np.float32)
    E = np.asarray(inputs["time_emb_landmark"], dtype=np.float32)
    L = np.asarray(inputs["time_landmark"], dtype=np.float32)
    in_maps = []
    for r in range(N_CORES):
        sl = slice(r * BL, (r + 1) * BL)
        in_maps.append({
            "z": np.ascontiguousarray(z[sl]),
            "t": np.ascontiguousarray(t[sl]).reshape(NT, P, 1),
            "e": np.ascontiguousarray(e[sl]).reshape(NT, P, 1),
            "log_tau": lt.reshape(1, 1),
            "emb": np.ascontiguousarray(E),
            "lm": np.ascontiguousarray(L).reshape(1, M),
        })
    return in_maps


_cached_nc = None


def kernel(**inputs) -> np.ndarray:
    global _cached_nc
    if _cached_nc is None:
        _cached_nc = build()
    in_maps = make_in_maps(inputs)
    res = run_bass_kernel_spmd(_cached_nc, in_maps,
                               core_ids=list(range(N_CORES)), trace=False)
    return np.concatenate(
        [res.results[r]["out"] for r in range(N_CORES)], axis=0
    ).astype(np.float32)
